# revision 22
# baseline (speedup 1.0000x reference)
"""Trainium2 Bass kernel for 4D convolution (3x3x3x3, pad 1, stride 1),
Winograd F(2,3) along the w axis.

Problem: x (2, 8, 7, 7, 48, 48) f32, conv (8, 648) f32, bias (8,) f32
         -> out (2, 8, 7, 7, 48, 48) f32.

Sharding: 8 cores = (batch b in {0,1}) x (h-chunk hc in {0..3}, 12 rows).

The w-axis 3-tap conv is computed as Winograd F(2,3): 24 tiles of 2
outputs each, 4 transformed points per tile.  The host precomputes the
input transform x_hat[q] (B^T-combinations of w-neighbours, pure input
marshaling) and the weight transform (G-combinations over i3).  The
device then runs, per (u, i0, i1, q, half), a banded matmul K=64
(8 h-window rows x 8 ch), M=48 (6 output h-rows x 8 out-ch), N=168
(7 v x 24 w-tiles), halves on disjoint PE row-group pairs streaming
concurrently (the proven 2-half choreography).  Columns streamed per
(u,i0,i1): 4 x 168 = 672 vs 3 x 336 = 1008 spatial -> 2/3 the PE time.

PSUM per u: [128, 2048]: half h banks 2h..2h+1, point q at col offset
h*1024 + (q//2)*512 + (q%2)*168 — uniform (2,2,168) nested strides, so
one ACT (h0) + one DVE (h1) op evacuates all 4 q-planes per half.
The inverse transform out_even = y0+y1+y2, out_odd = y1-y2-y3 and the
bias add run on the host.
"""

import sys

if "/opt/trn_rl_repo" not in sys.path:
    sys.path.insert(0, "/opt/trn_rl_repo")

import numpy as np
import ml_dtypes

B, C, OC = 2, 8, 8
U, V, H, W = 7, 7, 48, 48
TH = 12
THH = 6             # rows per half
S = TH + 2
SH = THH + 2        # window rows per half
KH = C * SH         # 64  contraction per half
MH = OC * THH       # 48  outputs per half
NCHUNKS = H // TH
NCORES = B * NCHUNKS
NT = W // 2         # 24 w-tiles
NQ = 4              # winograd points
NW = V * NT         # 168 columns per point-matmul
XROW = (V + 2) * NQ * NT  # 864 free elems per (partition, u-window)
XFREE = U * XROW

W_ORDER = [(i0, i1) for i0 in (1, 2, 0) for i1 in range(3)]

_built = {}


def _build_nc(reps=None):
    import contextlib

    import concourse.bacc as bacc
    import concourse.mybir as mybir
    from concourse.tile import TileContext

    BF16 = mybir.dt.bfloat16
    F32 = mybir.dt.float32

    nc = bacc.Bacc(
        "TRN2", target_bir_lowering=False, debug=False, num_devices=NCORES
    )
    xw_d = nc.dram_tensor("xw", [128, XFREE], BF16, kind="ExternalInput")
    wt_d = nc.dram_tensor("wt", [128, 36 * MH], BF16, kind="ExternalInput")
    out_d = nc.dram_tensor("out", [128, 4 * NQ * 2 * NW], BF16,
                           kind="ExternalOutput")

    with TileContext(nc) as tc:
        with (
            tc.tile_pool(name="sbuf", bufs=1) as pool,
            tc.tile_pool(name="psum", bufs=1, space="PSUM") as pp,
        ):
            loop = tc.For_i(0, reps, 1) if reps is not None else contextlib.nullcontext()
            with loop:
                # psum: one tile, all 8 banks: bank (h,q) at 2048h + 512q
                # holds the u-pair's two planes (u-even @+0, u-odd @+168)
                ps = pp.tile([128, 4096], F32, tag="ps", bufs=1, name="ps")
                scr = pool.tile([128, 64], BF16, tag="scr")
                nc.gpsimd.memset(scr[:], 0.0)
                # warmup into spare cols 336-399 of bank 0
                for _ in range(2):
                    nc.tensor.matmul(
                        ps[0:32, 336:400], scr[0:32, :32],
                        scr[0:32, :64], start=True, stop=True,
                        tile_position=(0, 0),
                    )

                w_sb = pool.tile([128, 36 * MH], BF16, tag="w", name="w_sb")
                x_sb = pool.tile([128, XFREE], BF16, tag="x", name="x_sb")
                o_sb = pool.tile([128, 4 * NQ * 2 * NW], BF16, tag="o",
                                 name="o_sb")
                nc.sync.dma_start(
                    out=x_sb[:, 0 : 2 * XROW], in_=xw_d[:, 0 : 2 * XROW]
                )
                nc.gpsimd.dma_start(
                    out=w_sb[:, 0 : 12 * MH], in_=wt_d[:, 0 : 12 * MH]
                )
                nc.sync.dma_start(
                    out=x_sb[:, 2 * XROW : 5 * XROW],
                    in_=xw_d[:, 2 * XROW : 5 * XROW],
                )
                nc.gpsimd.dma_start(
                    out=w_sb[:, 12 * MH :], in_=wt_d[:, 12 * MH :]
                )
                nc.sync.dma_start(
                    out=x_sb[:, 5 * XROW :], in_=xw_d[:, 5 * XROW :]
                )

                def lhsT_for(pos9, q, h):
                    rows = slice(64 * h, 64 * h + KH)
                    blk = (pos9 * NQ + q) * MH
                    return w_sb[rows, blk : blk + MH]

                def rhs_single(u, i0, i1, q, h):
                    return (
                        x_sb[
                            64 * h : 64 * h + KH,
                            (u + i0 - 1) * XROW : (u + i0) * XROW,
                        ]
                        .rearrange("p (v q w) -> p v q w", v=V + 2, q=NQ)
                        [:, i1 : i1 + V, q, :]
                    )

                def rhs_pair(w0, i1, q, h):
                    # two consecutive u-windows in one 3-free-dim AP
                    return (
                        x_sb[
                            64 * h : 64 * h + KH,
                            w0 * XROW : (w0 + 2) * XROW,
                        ]
                        .rearrange(
                            "p (n v q w) -> p n v q w", n=2, v=V + 2, q=NQ
                        )
                        [:, :, i1 : i1 + V, q, :]
                    )

                def pcol(h, q):
                    return 2048 * h + 512 * q

                PAIRS = [(0, 1), (2, 3), (4, 5), (6,)]
                for pr, us in enumerate(PAIRS):
                    shifts = [
                        (pos9, i0, i1)
                        for pos9, (i0, i1) in enumerate(W_ORDER)
                        if any(1 <= u + i0 <= 7 for u in us)
                    ]
                    n = len(shifts)
                    for idx, (pos9, i0, i1) in enumerate(shifts):
                        valid = [u for u in us if 1 <= u + i0 <= 7]
                        for q in range(NQ):
                            for h in range(2):
                                lhsT = lhsT_for(pos9, q, h)
                                if len(valid) == 2:
                                    out = ps[
                                        64 * h : 64 * h + MH,
                                        pcol(h, q) : pcol(h, q) + 2 * NW,
                                    ].rearrange("p (n w) -> p n w", n=2)
                                    rhs = rhs_pair(
                                        us[0] + i0 - 1, i1, q, h
                                    )
                                else:
                                    u = valid[0]
                                    off = pcol(h, q) + 168 * (u - us[0])
                                    out = ps[
                                        64 * h : 64 * h + MH,
                                        off : off + NW,
                                    ]
                                    rhs = rhs_single(u, i0, i1, q, h)
                                nc.tensor.matmul(
                                    out, lhsT, rhs,
                                    # start clears the whole PSUM bank:
                                    # only the bank's first MM sets it;
                                    # later planes init via has_written
                                    start=(idx == 0),
                                    stop=(idx == n - 1),
                                    skip_group_check=True,
                                )
                    # evacuate the pair: one strided op per half
                    prcols = slice(pr * NQ * 2 * NW, (pr + 1) * NQ * 2 * NW)
                    nc.scalar.activation(
                        out=o_sb[0:MH, prcols].rearrange(
                            "p (a w) -> p a w", a=NQ
                        ),
                        in_=ps[0:MH, 0:2048]
                        .rearrange("p (a w) -> p a w", a=NQ)[:, :, 0 : 2 * NW],
                        func=mybir.ActivationFunctionType.Identity,
                    )
                    nc.vector.tensor_scalar_add(
                        out=o_sb[64 : 64 + MH, prcols].rearrange(
                            "p (a w) -> p a w", a=NQ
                        ),
                        in0=ps[64 : 64 + MH, 2048:4096]
                        .rearrange("p (a w) -> p a w", a=NQ)[:, :, 0 : 2 * NW],
                        scalar1=0.0,
                    )
                    ring = nc.sync if pr % 2 == 0 else nc.gpsimd
                    ring.dma_start(
                        out=out_d[:, prcols], in_=o_sb[:, prcols]
                    )

    nc.compile()
    return nc


def _get_nc():
    if "nc" not in _built:
        _built["nc"] = _build_nc()
    return _built["nc"]


def _build_weight_inputs(conv):
    Wr = conv.reshape(OC, 3, 3, 3, 3, C).astype(np.float32)
    # G-transform over i3: g_hat[q] per (o, i0, i1, i2, c)
    g0, g1, g2 = Wr[..., 0, :], Wr[..., 1, :], Wr[..., 2, :]
    gh = [g0, (g0 + g1 + g2) / 2, (g0 - g1 + g2) / 2, g2]
    # wt[64h + s_rel*8 + c, (pos9*4 + q)*48 + t_rel*8 + o]
    wt = np.zeros((128, 36, MH), np.float32)
    for half in range(2):
        for t_rel in range(THH):
            for d in range(3):
                s_rel = t_rel + d
                if s_rel >= SH:
                    continue
                for pos9, (i0, i1) in enumerate(W_ORDER):
                    for q in range(NQ):
                        p0 = 64 * half + s_rel * 8
                        wt[p0 : p0 + 8, pos9 * NQ + q,
                           t_rel * 8 : t_rel * 8 + 8] = gh[q][
                            :, i0, i1, d, :
                        ].T
    return np.ascontiguousarray(
        wt.reshape(128, 36 * MH).astype(ml_dtypes.bfloat16)
    )


def _build_x_inputs(x):
    xh = np.zeros((B, C, U, V, H + 2, W), np.float32)
    xh[:, :, :, :, 1 : H + 1, :] = x
    xs = []
    for core in range(NCORES):
        b, hc = divmod(core, NCHUNKS)
        slab = xh[b, :, :, :, hc * TH : hc * TH + S, :]  # (C,U,V,14,W)
        xc = np.zeros((C, S, U, V + 2, W + 2), np.float32)
        xc[:, :, :, 1 : V + 1, 1 : W + 1] = slab.transpose(0, 3, 1, 2, 4)
        # forward Winograd transform along w (padded idx: w = idx - 1)
        # tile k inputs d = xpad[2k .. 2k+3]
        d0 = xc[..., 0:48:2]
        d1 = xc[..., 1:49:2]
        d2 = xc[..., 2:50:2]
        d3 = xc[..., 3:50:2]
        xq = np.stack(
            [d0 - d2, d1 + d2, d2 - d1, d1 - d3], axis=-2
        )  # (C, S, U, V+2, 4, 24)
        sm = xq.transpose(1, 0, 2, 3, 4, 5)  # (S, C, U, V+2, 4, 24)
        x128 = np.empty((128, XFREE), np.float32)
        x128[0:64] = sm[0:SH].reshape(KH, XFREE)
        x128[64:128] = sm[THH : THH + SH].reshape(KH, XFREE)
        xs.append(
            np.ascontiguousarray(x128.astype(ml_dtypes.bfloat16))
        )
    return xs


def kernel(x, conv, bias):
    from concourse.bass_utils import run_bass_kernel_spmd

    nc = _get_nc()
    wt = _build_weight_inputs(np.asarray(conv))
    xs = _build_x_inputs(np.asarray(x, dtype=np.float32))
    in_maps = [{"xw": xc, "wt": wt} for xc in xs]
    res = run_bass_kernel_spmd(nc, in_maps, core_ids=list(range(NCORES)))

    bias = np.asarray(bias, dtype=np.float32)
    out = np.empty((B, OC, U, V, H, W), np.float32)
    for core in range(NCORES):
        b, hc = divmod(core, NCHUNKS)
        raw = np.asarray(res.results[core]["out"], dtype=np.float32)
        # rows: half*64 + t_rel*8 + o ; cols: (pair, q, upar, v, k)
        raw = raw.reshape(128, 4, NQ, 2, V, NT)
        yy = np.concatenate([raw[0:MH], raw[64 : 64 + MH]], axis=0)
        yy = yy.reshape(TH, OC, 4, NQ, 2, V, NT)
        # u = 2*pair + upar
        y = yy.transpose(0, 1, 2, 4, 3, 5, 6).reshape(
            TH, OC, 8, NQ, V, NT
        )[:, :, :U]
        ev = y[:, :, :, 0] + y[:, :, :, 1] + y[:, :, :, 2]
        od = y[:, :, :, 1] - y[:, :, :, 2] - y[:, :, :, 3]
        w2 = np.stack([ev, od], axis=-1).reshape(TH, OC, U, V, W)
        out[b, :, :, :, hc * TH : (hc + 1) * TH, :] = (
            w2.transpose(1, 2, 3, 0, 4) + bias[:, None, None, None, None]
        )
    return out
